# revision 37
# baseline (speedup 1.0000x reference)
"""Trainium2 Bass kernel for nn_MultiHeadAttention (B=2, S=2048, D=1024, H=16).

Sharding: 8 cores = 2 batches x 4 head-groups. Core c handles batch c//4 and
heads [4*(c%4), 4*(c%4)+4); the host sums the 4 partial outputs per batch and
adds the output bias.

Per-core dataflow (ACT-paced, flipped attention):
  - qT/kT in [head_dim, seq] layout (2 heads per 128-partition tile);
    v in [kv, d] layout with a ones column per head ([v | 1] blocks of 65).
  - scoresT[kv, q] = kT.T @ qT per (head, kv-pair, 512q chunk) into a
    [128, 1024] PSUM tile; exp on ScalarE (scale=1/8) into bf16 SBUF. The
    exp stream (~128us) is the bottleneck engine; all other work is emitted
    through a budget-aware filler scheduler that spends the PE's ~500ns of
    slack per exp period without ever delaying the scores matmuls.
  - attn[q, d+1] = ex.T @ [v | 1] with the ex tile as the stationary operand:
    per (head, q-tile) a [128, 65] PSUM accumulator over the 16 kv tiles
    (N=65 per matmul instead of N=512 in the [d, q] orientation - half the
    PE cycles of the baseline scheme; col 64 collects the softmax
    denominator for free).
  - normalize: DVE reciprocal of the 4 sums columns + per-partition
    tensor_scalar multiply into bf16 (q is the partition dim, so no
    broadcast matmul is needed).
  - transpose [q, hd] -> [hd, q] via the DMA XBAR (zero PE cost), head pairs
    packed to 128 partitions so the output projection contracts K=128:
    out[q, D] accumulates 2 head-pair matmuls per 512-col half.
All matmuls bf16 with fp32 PSUM accumulation.
"""

import sys

for _p in ("/opt/trn_rl_repo",):
    if _p not in sys.path:
        sys.path.insert(0, _p)

import numpy as np
import ml_dtypes

BF16 = ml_dtypes.bfloat16

S = 2048          # sequence length
D = 1024          # embed dim
HC = 4            # heads per core
HD = 64           # head dim
DC = HC * HD      # per-core projection width (256)
ST = S // 128     # s-tiles (16)
DT = D // 128     # D-tiles (8)
QC = S // 512     # q-chunks of 512 (4)
NCORES = 8

_PROGRAM = None


def _build_program():
    import concourse.mybir as mybir
    import concourse.tile as tile
    from concourse import bacc

    dt = mybir.dt
    AF = mybir.ActivationFunctionType
    ALU = mybir.AluOpType

    nc = bacc.Bacc()

    xqT = nc.declare_dram_parameter("xqT", [D, S], dt.bfloat16, isOutput=False)
    xkT = nc.declare_dram_parameter("xkT", [D, S], dt.bfloat16, isOutput=False)
    xvT = nc.declare_dram_parameter("xvT", [D, S], dt.bfloat16, isOutput=False)
    wq = nc.declare_dram_parameter("wq", [D, DC], dt.bfloat16, isOutput=False)
    wk = nc.declare_dram_parameter("wk", [D, DC], dt.bfloat16, isOutput=False)
    wv = nc.declare_dram_parameter("wv", [D, DC], dt.bfloat16, isOutput=False)
    wo2 = nc.declare_dram_parameter("wo2", [128, 2, D], dt.bfloat16, isOutput=False)
    bq = nc.declare_dram_parameter("bq", [128, 2], dt.float32, isOutput=False)
    bk = nc.declare_dram_parameter("bk", [128, 2], dt.float32, isOutput=False)
    bv = nc.declare_dram_parameter("bv", [128, DC], dt.float32, isOutput=False)
    ident = nc.declare_dram_parameter("ident", [128, 128], dt.bfloat16,
                                      isOutput=False)
    out = nc.declare_dram_parameter("out", [S, D], dt.float32, isOutput=True)

    out_t = out.rearrange("(t p) d -> t p d", p=128)
    xqr = xqT.rearrange("(t p) s -> p t s", p=128)
    xkr = xkT.rearrange("(t p) s -> p t s", p=128)
    xvr = xvT.rearrange("(t p) s -> p t s", p=128)

    with tile.TileContext(nc) as tc:
        with (
            tc.tile_pool(name="const", bufs=1) as cp,
            tc.tile_pool(name="x5", bufs=16) as x5,    # 512-col x tiles
            tc.tile_pool(name="xh", bufs=24) as xh,    # 1024-col x tiles
            tc.tile_pool(name="expp", bufs=30) as ep,
            tc.tile_pool(name="aq", bufs=6) as aqp,
            tc.tile_pool(name="rc", bufs=4) as rcp,
            tc.tile_pool(name="atp", bufs=3) as atp,
            tc.tile_pool(name="outp", bufs=6) as op_,
            tc.tile_pool(name="pa", bufs=2, space="PSUM") as pa,
            tc.tile_pool(name="ps", bufs=3, space="PSUM") as psp,
            tc.tile_pool(name="pt", bufs=1, space="PSUM") as ptp,
        ):
            # ---- constants ----
            wq_sb = cp.tile([128, DT, DC], dt.bfloat16, tag="wq_sb")
            wk_sb = cp.tile([128, DT, DC], dt.bfloat16, tag="wk_sb")
            wv_sb = cp.tile([128, DT, DC], dt.bfloat16, tag="wv_sb")
            wo2_sb = cp.tile([128, 2, D], dt.bfloat16, tag="wo2_sb")
            bq_sb = cp.tile([128, 2], dt.float32, tag="bq_sb")
            bk_sb = cp.tile([128, 2], dt.float32, tag="bk_sb")
            bv_sb = cp.tile([128, DC], dt.float32, tag="bv_sb")
            v_sb = cp.tile([128, ST, HC * 65], dt.bfloat16, tag="v_sb")
            ident_sb = cp.tile([128, 128], dt.bfloat16, tag="ident_sb")
            dum = cp.tile([1, 4], dt.bfloat16, tag="dum")
            qT_sb = [cp.tile([128, 2, 512], dt.bfloat16, tag=f"qT_sb{i}",
                             name=f"qT_sb{i}") for i in range(QC)]
            kT_sb = [cp.tile([128, 2, 512], dt.bfloat16, tag=f"kT_sb{i}",
                             name=f"kT_sb{i}") for i in range(QC)]

            # ones columns for the softmax denominators (Pool engine, t~0),
            # and a dummy exp to hoist the ACT table load off the exp stream.
            nc.gpsimd.memset(v_sb[:], 1.0)
            nc.vector.memset(dum[:], 0.0)
            nc.scalar.activation(dum[:, 2:4], dum[:, 0:2], AF.Exp)

            # ---- DMA prologue: order sets the single DMA engine's schedule.
            # kT/qT chunk0 first (exp stream starts ~10.5us), then the rest in
            # deadline order: xk c1 (kvb2-3), xk half1 (kvb4-7), xv h0 + xq c1
            # (attention / qc1), xv h1, wo2, xq c2/c3 (late q-chunks).
            nc.sync.dma_start(wk_sb[:], wk.rearrange("(t p) m -> p t m", p=128))
            nc.sync.dma_start(bk_sb[:], bk[:])
            nc.sync.dma_start(bq_sb[:], bq[:])

            xk_c0 = [None] * DT
            xk_c1 = [None] * DT
            xk_h1 = [None] * DT
            xq_q0 = [None] * DT
            xq_c1 = [None] * DT
            xq_h1 = [None] * DT
            xv_h = [[None] * DT for _ in range(2)]

            def load(pool, lst, src, cols, nm, w):
                for Dti in range(DT):
                    t = pool.tile([128, w], dt.bfloat16, tag=pool.name,
                                  name=f"{nm}_{Dti}")
                    nc.sync.dma_start(t[:], src[:, Dti, cols[0]:cols[1]])
                    lst[Dti] = t

            load(x5, xk_c0, xkr, (0, 512), "xk_c0", 512)
            nc.sync.dma_start(wq_sb[:], wq.rearrange("(t p) m -> p t m", p=128))
            load(x5, xq_q0, xqr, (0, 512), "xq_q0", 512)
            load(x5, xk_c1, xkr, (512, 1024), "xk_c1", 512)
            load(xh, xk_h1, xkr, (1024, 2048), "xk_h1", 1024)
            nc.sync.dma_start(wv_sb[:], wv.rearrange("(t p) m -> p t m", p=128))
            nc.sync.dma_start(bv_sb[:], bv[:])
            load(xh, xv_h[0], xvr, (0, 1024), "xv0", 1024)
            load(x5, xq_c1, xqr, (512, 1024), "xq_c1", 512)
            load(xh, xv_h[1], xvr, (1024, 2048), "xv1", 1024)
            nc.sync.dma_start(wo2_sb[:], wo2[:])
            load(xh, xq_h1, xqr, (1024, 2048), "xq_h1", 1024)
            nc.sync.dma_start(ident_sb[:], ident[:])

            k_rhs = [lambda D_, t=xk_c0: t[D_][:],
                     lambda D_, t=xk_c1: t[D_][:],
                     lambda D_, t=xk_h1: t[D_][:, 0:512],
                     lambda D_, t=xk_h1: t[D_][:, 512:1024]]
            q_rhs = [lambda D_, t=xq_q0: t[D_][:],
                     lambda D_, t=xq_c1: t[D_][:],
                     lambda D_, t=xq_h1: t[D_][:, 0:512],
                     lambda D_, t=xq_h1: t[D_][:, 512:1024]]

            # full-pt (N=512) projection group: ~1.7us of PE
            def qk_gran(rhs_of, w_sb, dst, b_sb, c, pt, pool):
                ps = pool.tile([128, 512], dt.float32, tag=pool.name,
                               name=f"pg_{dst[c].tensor.name}_{pt}")
                for Dti in range(DT):
                    nc.tensor.matmul(
                        ps[:],
                        w_sb[:, Dti, pt * 128:(pt + 1) * 128],
                        rhs_of(Dti),
                        start=(Dti == 0),
                        stop=(Dti == DT - 1),
                    )
                nc.vector.tensor_scalar_add(
                    dst[c][:, pt, :], ps[:], b_sb[:, pt:pt + 1],
                )

            vp_done = [0]     # number of v s-tiles fully emitted

            def v_proj2(st2):
                ps = ptp.tile([128, 2, DC], dt.float32, tag="pt",
                              name=f"vp_{st2}")
                for u in range(2):
                    st = 2 * st2 + u
                    half, off = st // 8, (st % 8) * 128
                    for Dti in range(DT):
                        nc.tensor.matmul(
                            ps[:, u, :],
                            xv_h[half][Dti][:, off:off + 128],
                            wv_sb[:, Dti, :],
                            start=(Dti == 0),
                            stop=(Dti == DT - 1),
                        )
                for u in range(2):
                    st = 2 * st2 + u
                    nc.vector.tensor_tensor(
                        v_sb[:, st, :].rearrange("p (h c) -> p h c", c=65)[:, :, 0:64],
                        ps[:, u, :].rearrange("p (h d) -> p h d", d=HD),
                        bv_sb.rearrange("p (h d) -> p h d", d=HD),
                        ALU.add,
                    )
                vp_done[0] = 2 * st2 + 2

            # ---- attention stream pieces ----
            psS = {}      # qc -> [128, 16] sums accumulator
            psA = {}      # qc -> [2 psum accumulator banks of 8 cols each]
            atT = {}      # qc -> transposed normalized attn [128 hd, 2 hp, 512 q]

            def scores_exp(qc, kvb, h):
                pt, lo = h // 2, (h % 2) * 64
                scp = pa.tile([128, 1024], dt.float32, tag="pa",
                              name=f"sc_{qc}_{kvb}_{h}")
                for j in range(2):
                    kt = kvb * 2 + j
                    nc.tensor.matmul(
                        scp[:, j * 512:(j + 1) * 512],
                        kT_sb[kt // 4][lo:lo + 64, pt, (kt % 4) * 128:(kt % 4 + 1) * 128],
                        qT_sb[qc][lo:lo + 64, pt, :],
                        start=True,
                        stop=True,
                    )
                ex = ep.tile([128, 1024], dt.bfloat16, tag="ex",
                             name=f"ex_{qc}_{kvb}_{h}")
                nc.scalar.activation(ex[:], scp[:], AF.Exp, scale=0.125)
                return ex

            def attn(qc, kvb, h, ex):
                if qc not in psA:
                    psA[qc] = [psp.tile([128, 8, HD], dt.float32, tag="ps",
                                        name=f"att_{qc}_{b}") for b in range(2)]
                    psS[qc] = psp.tile([128, 16], dt.float32, tag="ps",
                                       name=f"asum_{qc}")
                # start=True zeroes the whole 2KB PSUM bank, so with several
                # accumulation groups per bank only the very first write into
                # each bank may carry start; everything else accumulates.
                for j in range(2):
                    kt = kvb * 2 + j
                    first = kvb == 0 and j == 0 and h == 0
                    last = kvb == 7 and j == 1 and h == HC - 1
                    for qt in range(4):
                        ex_sl = ex[:, j * 512 + qt * 128: j * 512 + (qt + 1) * 128]
                        nc.tensor.matmul(
                            psA[qc][qt // 2][:, (qt % 2) * 4 + h, :],
                            ex_sl,
                            v_sb[:, kt, h * 65:h * 65 + 64],
                            start=(first and qt % 2 == 0),
                            stop=(last and qt % 2 == 1),
                            skip_group_check=True,
                        )
                        c = qt * 4 + h
                        nc.tensor.matmul(
                            psS[qc][:, c:c + 1],
                            ex_sl,
                            v_sb[:, kt, h * 65 + 64:h * 65 + 65],
                            start=(first and qt == 0),
                            stop=(last and qt == 3),
                            skip_group_check=True,
                        )

            def tail_norm(qc):
                at = atp.tile([128, 2, 512], dt.bfloat16, tag="at", name=f"atT_{qc}")
                aqs = []
                for qt in range(4):
                    rc = rcp.tile([128, HC], dt.float32, tag="rc",
                                  name=f"rc_{qc}_{qt}")
                    nc.vector.reciprocal(rc[:], psS[qc][:, qt * 4:qt * 4 + 4])
                    aq_t = aqp.tile([128, DC], dt.bfloat16, tag="aq",
                                    name=f"aq_{qc}_{qt}")
                    for h in range(HC):
                        nc.vector.tensor_scalar_mul(
                            aq_t[:, h * 64:(h + 1) * 64],
                            psA[qc][qt // 2][:, (qt % 2) * 4 + h, :],
                            rc[:, h:h + 1],
                        )
                    if not drain_mode[0]:
                        for hp in range(2):
                            nc.sync.dma_start_transpose(
                                at[:, hp, qt * 128:(qt + 1) * 128],
                                aq_t[:, hp * 128:(hp + 1) * 128],
                            )
                    else:
                        aqs.append(aq_t)
                if drain_mode[0]:
                    # tail: PE is idle and HWDGE is busy with out-DMAs, so
                    # transpose via the PE (identity matmul) and copy the
                    # bf16 PSUM result back on the idle Pool engine
                    for qt in range(4):
                        for hp in range(2):
                            trp = pa.tile([128, 128], dt.bfloat16, tag="pa",
                                          name=f"trp_{qt}_{hp}")
                            nc.tensor.transpose(
                                trp[:], aqs[qt][:, hp * 128:(hp + 1) * 128],
                                ident_sb[:])
                            nc.vector.tensor_copy(
                                at[:, hp, qt * 128:(qt + 1) * 128], trp[:])
                del psA[qc]
                del psS[qc]
                atT[qc] = at

            def po_half(qc, qt, dc2, pool):
                poh = pool.tile([128, 512], dt.float32, tag=pool.name,
                                name=f"po_{qc}_{qt}_{dc2}")
                for hp in range(2):
                    nc.tensor.matmul(
                        poh[:],
                        atT[qc][:, hp, qt * 128:(qt + 1) * 128],
                        wo2_sb[:, hp, dc2 * 512:(dc2 + 1) * 512],
                        start=(hp == 0),
                        stop=(hp == 1),
                    )
                o_sb = op_.tile([128, 512], dt.float32, tag="o",
                                name=f"o_{qc}_{qt}_{dc2}")
                nc.vector.tensor_copy(o_sb[:], poh[:])
                nc.sync.dma_start(out_t[qc * 4 + qt][:, dc2 * 512:(dc2 + 1) * 512],
                                  o_sb[:])

            # ---- prologue PE work: pt0 pair only; the first two stream
            # tiles (h0/h1 read pt0) are emitted before the pt1 pair so their
            # exps do not absorb the pt1 projections into their waits
            qk_gran(k_rhs[0], wk_sb, kT_sb, bk_sb, 0, 0, pa)
            qk_gran(q_rhs[0], wq_sb, qT_sb, bq_sb, 0, 0, pa)

            # ---- filler queues for the budget scheduler ----
            # (nb = earliest stream-tile index; chosen from the DMA schedule)
            def kg(c, pt):
                return lambda: qk_gran(k_rhs[c], wk_sb, kT_sb, bk_sb, c, pt, ptp)

            def qg(c, pt):
                return lambda: qk_gran(q_rhs[c], wq_sb, qT_sb, bq_sb, c, pt, ptp)

            _LOG = []   # scheduler trace (debug)
            projq = [(2, 7, 1712.0, kg(1, 0)), (3, 9, 1712.0, kg(1, 1)),
                     (7, 15, 1712.0, kg(2, 0)), (8, 17, 1712.0, kg(2, 1)),
                     (11, 23, 1712.0, kg(3, 0)), (12, 25, 1712.0, kg(3, 1)),
                     (17, 31, 1712.0, qg(1, 0)), (18, 33, 1712.0, qg(1, 1)),
                     (40, 63, 1712.0, qg(2, 0)), (42, 65, 1712.0, qg(2, 1)),
                     (44, 95, 1712.0, qg(3, 0)), (46, 97, 1712.0, qg(3, 1))]
            vpq = [(12 if st2 < 4 else 21, 1712.0, st2) for st2 in range(ST // 2)]
            poq = []
            po_n = [0]
            attnq = []   # ("attn", nb, qc, kvb, h, ex) / ("tail", nb, qc) /
                         # ("po", nb, qc, qt, dc2)
            n_attn_emitted = [0]
            tr_free = [0]        # next stream idx the transient ring is free
            last_tail = [-10]    # stream idx of the last tail_norm pop
            drain_mode = [False]

            def attn_ready(item, idx):
                kind = item[0]
                if item[1] > idx:
                    return False
                if kind == "attn":
                    if item[2] not in psA and idx < last_tail[0] + 3:
                        return False
                    return vp_done[0] >= min(2 * item[3] + 2, ST)
                return True

            def pop_attn(idx):
                item = attnq.pop(0)
                _LOG.append((idx, item[0]))
                if item[0] == "attn":
                    _, _, qc, kvb, h, ex = item
                    attn(qc, kvb, h, ex)
                    n_attn_emitted[0] += 1
                    return 230.0
                qc = item[2]
                tail_norm(qc)
                last_tail[0] = idx
                for k, (qt, dc2) in enumerate(
                        (q, d) for q in range(4) for d in range(2)):
                    poq.append((idx + 4 + 2 * k, qc, qt, dc2))
                return 0.0

            def pop_po(idx):
                nb, qc, qt, dc2 = poq.pop(0)
                _LOG.append((idx, "po"))
                if drain_mode[0]:
                    pool = pa if po_n[0] % 2 == 0 else ptp
                else:
                    pool = ptp
                po_n[0] += 1
                po_half(qc, qt, dc2, pool)
                tr_free[0] = idx + 2
                return 426.0

            carry = [0.0]
            BUDGET, CAP, FORCE_AT = 550.0, 2600.0, 22

            def fillers(idx, force_at=FORCE_AT):
                carry[0] = min(carry[0] + BUDGET, CAP)
                while True:
                    pending = idx + 1 - n_attn_emitted[0]
                    tr_ok = idx >= tr_free[0]
                    if poq and tr_ok and idx >= poq[0][0] + 6:
                        carry[0] -= pop_po(idx)
                        continue
                    if pending >= force_at and attnq:
                        it = attnq[0]
                        if (it[0] == "attn"
                                and vp_done[0] < min(2 * it[3] + 2, ST) and vpq):
                            nb, cost, st2 = vpq.pop(0)
                            v_proj2(st2)
                            carry[0] -= cost
                            tr_free[0] = idx + 3
                            continue
                        if attn_ready(it, idx) or pending >= force_at + 6:
                            carry[0] -= pop_attn(idx)
                            continue
                    # cheap attn/tail pops first: their tiny matmuls shield
                    # the preceding scores tile's exp from absorbing a
                    # transient's matmuls into its coalesced semaphore wait
                    if attnq and attn_ready(attnq[0], idx):
                        kind = attnq[0][0]
                        if kind == "attn" and 230.0 <= carry[0]:
                            carry[0] -= pop_attn(idx)
                            continue
                        if kind == "tail":
                            carry[0] -= pop_attn(idx)
                            continue
                    if (projq and projq[0][0] <= idx
                            and projq[0][2] <= carry[0] and tr_ok):
                        nb, dl, cost, fn = projq.pop(0)
                        fn()
                        _LOG.append((idx, "proj"))
                        carry[0] -= cost
                        tr_free[0] = idx + 3
                        continue
                    if (vpq and vpq[0][0] <= idx
                            and vpq[0][1] <= carry[0] and tr_ok):
                        nb, cost, st2 = vpq.pop(0)
                        v_proj2(st2)
                        carry[0] -= cost
                        tr_free[0] = idx + 3
                        continue
                    if poq and poq[0][0] <= idx and 426.0 <= carry[0] and tr_ok:
                        carry[0] -= pop_po(idx)
                        continue
                    break

            # ---- the exp stream with interleaved fillers ----
            def prologue_pt1(idx):
                qk_gran(k_rhs[0], wk_sb, kT_sb, bk_sb, 0, 1, ptp)
                qk_gran(q_rhs[0], wq_sb, qT_sb, bq_sb, 0, 1, ptp)
                tr_free[0] = idx + 4

            idx = 0
            for qc in range(QC):
                for kvb in range(8):
                    for h in range(HC):
                        if idx == 2:
                            prologue_pt1(idx)
                        # hard deadline: a projection chunk must be emitted
                        # before the first scores tile that reads it
                        while projq and projq[0][1] <= idx:
                            nb, dl, cost, fn = projq.pop(0)
                            fn()
                            carry[0] -= cost
                            tr_free[0] = idx + 3
                        ex = scores_exp(qc, kvb, h)
                        attnq.append(("attn", idx + 1, qc, kvb, h, ex))
                        if kvb == 7 and h == HC - 1:
                            attnq.append(("tail", idx + 2, qc))
                        fa = FORCE_AT
                        if qc == QC - 1:
                            fa = max(2, FORCE_AT - max(0, idx - 96))
                        fillers(idx, fa)
                        idx += 1

            # ---- drain: remaining attn, last tail, last po (pa ring is free
            # now, so po ping-pongs through it instead of the 1-bank ring)
            drain_mode[0] = True
            _LOG.append(("DRAIN", [it[0] for it in attnq], len(poq),
                         len(projq), len(vpq)))
            while projq or vpq or attnq or poq:
                if projq:
                    projq.pop(0)[3]()
                    continue
                if vpq:
                    v_proj2(vpq.pop(0)[2])
                    continue
                if attnq and attn_ready(attnq[0], 10 ** 9):
                    pop_attn(idx)
                    idx += 1
                    continue
                if poq:
                    pop_po(idx)
                    idx += 1
                    continue
                raise RuntimeError("scheduler deadlock")

    nc.finalize()
    return nc


def _get_program():
    global _PROGRAM
    if _PROGRAM is None:
        _PROGRAM = _build_program()
    return _PROGRAM


def _prep_core_inputs(x_q, x_k, x_v, wq, bq, wk, bk, wv, bv, wo):
    """Build the 8 per-core input dicts (host-side shard + cast)."""
    xT = {}
    for b in range(2):
        xT[b] = (
            np.ascontiguousarray(x_q[b].T).astype(BF16),
            np.ascontiguousarray(x_k[b].T).astype(BF16),
            np.ascontiguousarray(x_v[b].T).astype(BF16),
        )
    in_maps = []
    for c in range(NCORES):
        b, g = c // 4, c % 4
        sl = slice(g * DC, (g + 1) * DC)
        # wo2[p, hp, :] = wo[g*DC + hp*128 + p, :] - head pairs stacked to 128
        # partitions, matching the transposed at_q column order.
        wo2_c = np.ascontiguousarray(
            wo[sl, :].reshape(2, 128, D).transpose(1, 0, 2)
        ).astype(BF16)
        in_maps.append({
            "xqT": xT[b][0],
            "xkT": xT[b][1],
            "xvT": xT[b][2],
            "wq": wq[:, sl].astype(BF16),
            "wk": wk[:, sl].astype(BF16),
            "wv": wv[:, sl].astype(BF16),
            "wo2": wo2_c,
            "bq": np.ascontiguousarray(bq[sl].reshape(2, 128).T).astype(np.float32),
            "bk": np.ascontiguousarray(bk[sl].reshape(2, 128).T).astype(np.float32),
            "bv": np.broadcast_to(bv[sl], (128, DC)).astype(np.float32).copy(),
            "ident": np.eye(128, dtype=BF16),
        })
    return in_maps


def kernel(x_q, x_k, x_v, wq, bq, wk, bk, wv, bv, wo, bo):
    from concourse.bass_utils import run_bass_kernel_spmd

    x_q = np.asarray(x_q, np.float32)
    x_k = np.asarray(x_k, np.float32)
    x_v = np.asarray(x_v, np.float32)
    wq = np.asarray(wq, np.float32)
    wk = np.asarray(wk, np.float32)
    wv = np.asarray(wv, np.float32)
    wo = np.asarray(wo, np.float32)
    bq = np.asarray(bq, np.float32)
    bk = np.asarray(bk, np.float32)
    bv = np.asarray(bv, np.float32)
    bo = np.asarray(bo, np.float32)

    nc = _get_program()
    in_maps = _prep_core_inputs(x_q, x_k, x_v, wq, bq, wk, bk, wv, bv, wo)
    res = run_bass_kernel_spmd(nc, in_maps, list(range(NCORES)))

    out = np.zeros((2, S, D), np.float32)
    for c in range(NCORES):
        out[c // 4] += res.results[c]["out"]
    out += bo
    return out


# revision 38
# speedup vs baseline: 1.0072x; 1.0072x over previous
"""Trainium2 Bass kernel for nn_MultiHeadAttention (B=2, S=2048, D=1024, H=16).

Sharding: 8 cores = 2 batches x 4 head-groups. Core c handles batch c//4 and
heads [4*(c%4), 4*(c%4)+4); the host sums the 4 partial outputs per batch and
adds the output bias.

Per-core dataflow (ACT-paced, flipped attention):
  - qT/kT in [head_dim, seq] layout (2 heads per 128-partition tile);
    v in [kv, d] layout with a ones column per head ([v | 1] blocks of 65).
  - scoresT[kv, q] = kT.T @ qT per (head, kv-pair, 512q chunk) into a
    [128, 1024] PSUM tile; exp on ScalarE (scale=1/8) into bf16 SBUF. The
    exp stream (~128us) is the bottleneck engine; all other work is emitted
    through a budget-aware filler scheduler that spends the PE's ~500ns of
    slack per exp period without ever delaying the scores matmuls.
  - attn[q, d+1] = ex.T @ [v | 1] with the ex tile as the stationary operand:
    per (head, q-tile) a [128, 65] PSUM accumulator over the 16 kv tiles
    (N=65 per matmul instead of N=512 in the [d, q] orientation - half the
    PE cycles of the baseline scheme; col 64 collects the softmax
    denominator for free).
  - normalize: DVE reciprocal of the 4 sums columns + per-partition
    tensor_scalar multiply into bf16 (q is the partition dim, so no
    broadcast matmul is needed).
  - transpose [q, hd] -> [hd, q] via the DMA XBAR (zero PE cost), head pairs
    packed to 128 partitions so the output projection contracts K=128:
    out[q, D] accumulates 2 head-pair matmuls per 512-col half.
All matmuls bf16 with fp32 PSUM accumulation.
"""

import sys

for _p in ("/opt/trn_rl_repo",):
    if _p not in sys.path:
        sys.path.insert(0, _p)

import numpy as np
import ml_dtypes

BF16 = ml_dtypes.bfloat16

S = 2048          # sequence length
D = 1024          # embed dim
HC = 4            # heads per core
HD = 64           # head dim
DC = HC * HD      # per-core projection width (256)
ST = S // 128     # s-tiles (16)
DT = D // 128     # D-tiles (8)
QC = S // 512     # q-chunks of 512 (4)
NCORES = 8

_PROGRAM = None


def _build_program():
    import concourse.mybir as mybir
    import concourse.tile as tile
    from concourse import bacc

    dt = mybir.dt
    AF = mybir.ActivationFunctionType
    ALU = mybir.AluOpType

    nc = bacc.Bacc()

    xqT = nc.declare_dram_parameter("xqT", [D, S], dt.bfloat16, isOutput=False)
    xkT = nc.declare_dram_parameter("xkT", [D, S], dt.bfloat16, isOutput=False)
    xvT = nc.declare_dram_parameter("xvT", [D, S], dt.bfloat16, isOutput=False)
    wq = nc.declare_dram_parameter("wq", [D, DC], dt.bfloat16, isOutput=False)
    wk = nc.declare_dram_parameter("wk", [D, DC], dt.bfloat16, isOutput=False)
    wv = nc.declare_dram_parameter("wv", [D, DC], dt.bfloat16, isOutput=False)
    wo2 = nc.declare_dram_parameter("wo2", [128, 2, D], dt.bfloat16, isOutput=False)
    bq = nc.declare_dram_parameter("bq", [128, 2], dt.float32, isOutput=False)
    bk = nc.declare_dram_parameter("bk", [128, 2], dt.float32, isOutput=False)
    bv = nc.declare_dram_parameter("bv", [128, DC], dt.float32, isOutput=False)
    ident = nc.declare_dram_parameter("ident", [128, 128], dt.bfloat16,
                                      isOutput=False)
    out = nc.declare_dram_parameter("out", [S, D], dt.float32, isOutput=True)

    out_t = out.rearrange("(t p) d -> t p d", p=128)
    xqr = xqT.rearrange("(t p) s -> p t s", p=128)
    xkr = xkT.rearrange("(t p) s -> p t s", p=128)
    xvr = xvT.rearrange("(t p) s -> p t s", p=128)

    with tile.TileContext(nc) as tc:
        with (
            tc.tile_pool(name="const", bufs=1) as cp,
            tc.tile_pool(name="x5", bufs=16) as x5,    # 512-col x tiles
            tc.tile_pool(name="xh", bufs=24) as xh,    # 1024-col x tiles
            tc.tile_pool(name="expp", bufs=30) as ep,
            tc.tile_pool(name="aq", bufs=6) as aqp,
            tc.tile_pool(name="rc", bufs=4) as rcp,
            tc.tile_pool(name="atp", bufs=3) as atp,
            tc.tile_pool(name="outp", bufs=6) as op_,
            tc.tile_pool(name="pa", bufs=2, space="PSUM") as pa,
            tc.tile_pool(name="ps", bufs=3, space="PSUM") as psp,
            tc.tile_pool(name="pt", bufs=1, space="PSUM") as ptp,
        ):
            # ---- constants ----
            wq_sb = cp.tile([128, DT, DC], dt.bfloat16, tag="wq_sb")
            wk_sb = cp.tile([128, DT, DC], dt.bfloat16, tag="wk_sb")
            wv_sb = cp.tile([128, DT, DC], dt.bfloat16, tag="wv_sb")
            wo2_sb = cp.tile([128, 2, D], dt.bfloat16, tag="wo2_sb")
            bq_sb = cp.tile([128, 2], dt.float32, tag="bq_sb")
            bk_sb = cp.tile([128, 2], dt.float32, tag="bk_sb")
            bv_sb = cp.tile([128, DC], dt.float32, tag="bv_sb")
            v_sb = cp.tile([128, ST, HC * 65], dt.bfloat16, tag="v_sb")
            ident_sb = cp.tile([128, 128], dt.bfloat16, tag="ident_sb")
            dum = cp.tile([1, 4], dt.bfloat16, tag="dum")
            qT_sb = [cp.tile([128, 2, 512], dt.bfloat16, tag=f"qT_sb{i}",
                             name=f"qT_sb{i}") for i in range(QC)]
            kT_sb = [cp.tile([128, 2, 512], dt.bfloat16, tag=f"kT_sb{i}",
                             name=f"kT_sb{i}") for i in range(QC)]

            # ones columns for the softmax denominators (Pool engine, t~0),
            # and a dummy exp to hoist the ACT table load off the exp stream.
            nc.gpsimd.memset(v_sb[:], 1.0)
            nc.vector.memset(dum[:], 0.0)
            nc.scalar.activation(dum[:, 2:4], dum[:, 0:2], AF.Exp)

            # ---- DMA prologue: order sets the single DMA engine's schedule.
            # kT/qT chunk0 first (exp stream starts ~10.5us), then the rest in
            # deadline order: xk c1 (kvb2-3), xk half1 (kvb4-7), xv h0 + xq c1
            # (attention / qc1), xv h1, wo2, xq c2/c3 (late q-chunks).
            nc.sync.dma_start(wk_sb[:], wk.rearrange("(t p) m -> p t m", p=128))
            nc.sync.dma_start(bk_sb[:], bk[:])
            nc.sync.dma_start(bq_sb[:], bq[:])

            xk_c0 = [None] * DT
            xk_c1 = [None] * DT
            xk_h1 = [None] * DT
            xq_q0 = [None] * DT
            xq_c1 = [None] * DT
            xq_h1 = [None] * DT
            xv_h = [[None] * DT for _ in range(2)]

            def load(pool, lst, src, cols, nm, w):
                for Dti in range(DT):
                    t = pool.tile([128, w], dt.bfloat16, tag=pool.name,
                                  name=f"{nm}_{Dti}")
                    nc.sync.dma_start(t[:], src[:, Dti, cols[0]:cols[1]])
                    lst[Dti] = t

            load(x5, xk_c0, xkr, (0, 512), "xk_c0", 512)
            nc.sync.dma_start(wq_sb[:], wq.rearrange("(t p) m -> p t m", p=128))
            load(x5, xq_q0, xqr, (0, 512), "xq_q0", 512)
            load(x5, xk_c1, xkr, (512, 1024), "xk_c1", 512)
            load(xh, xk_h1, xkr, (1024, 2048), "xk_h1", 1024)
            nc.sync.dma_start(wv_sb[:], wv.rearrange("(t p) m -> p t m", p=128))
            nc.sync.dma_start(bv_sb[:], bv[:])
            load(xh, xv_h[0], xvr, (0, 1024), "xv0", 1024)
            load(x5, xq_c1, xqr, (512, 1024), "xq_c1", 512)
            load(xh, xv_h[1], xvr, (1024, 2048), "xv1", 1024)
            nc.sync.dma_start(wo2_sb[:], wo2[:])
            load(xh, xq_h1, xqr, (1024, 2048), "xq_h1", 1024)
            nc.sync.dma_start(ident_sb[:], ident[:])

            k_rhs = [lambda D_, t=xk_c0: t[D_][:],
                     lambda D_, t=xk_c1: t[D_][:],
                     lambda D_, t=xk_h1: t[D_][:, 0:512],
                     lambda D_, t=xk_h1: t[D_][:, 512:1024]]
            q_rhs = [lambda D_, t=xq_q0: t[D_][:],
                     lambda D_, t=xq_c1: t[D_][:],
                     lambda D_, t=xq_h1: t[D_][:, 0:512],
                     lambda D_, t=xq_h1: t[D_][:, 512:1024]]

            # full-pt (N=512) projection group: ~1.7us of PE
            def qk_gran(rhs_of, w_sb, dst, b_sb, c, pt, pool):
                ps = pool.tile([128, 512], dt.float32, tag=pool.name,
                               name=f"pg_{dst[c].tensor.name}_{pt}")
                for Dti in range(DT):
                    nc.tensor.matmul(
                        ps[:],
                        w_sb[:, Dti, pt * 128:(pt + 1) * 128],
                        rhs_of(Dti),
                        start=(Dti == 0),
                        stop=(Dti == DT - 1),
                    )
                nc.vector.tensor_scalar_add(
                    dst[c][:, pt, :], ps[:], b_sb[:, pt:pt + 1],
                )

            vp_done = [0]     # number of v s-tiles fully emitted

            def v_proj2(st2):
                ps = ptp.tile([128, 2, DC], dt.float32, tag="pt",
                              name=f"vp_{st2}")
                for u in range(2):
                    st = 2 * st2 + u
                    half, off = st // 8, (st % 8) * 128
                    for Dti in range(DT):
                        nc.tensor.matmul(
                            ps[:, u, :],
                            xv_h[half][Dti][:, off:off + 128],
                            wv_sb[:, Dti, :],
                            start=(Dti == 0),
                            stop=(Dti == DT - 1),
                        )
                for u in range(2):
                    st = 2 * st2 + u
                    nc.vector.tensor_tensor(
                        v_sb[:, st, :].rearrange("p (h c) -> p h c", c=65)[:, :, 0:64],
                        ps[:, u, :].rearrange("p (h d) -> p h d", d=HD),
                        bv_sb.rearrange("p (h d) -> p h d", d=HD),
                        ALU.add,
                    )
                vp_done[0] = 2 * st2 + 2

            # ---- attention stream pieces ----
            psS = {}      # qc -> [128, 16] sums accumulator
            psA = {}      # qc -> [2 psum accumulator banks of 8 cols each]
            atT = {}      # qc -> transposed normalized attn [128 hd, 2 hp, 512 q]

            def scores_exp(qc, kvb, h):
                pt, lo = h // 2, (h % 2) * 64
                scp = pa.tile([128, 1024], dt.float32, tag="pa",
                              name=f"sc_{qc}_{kvb}_{h}")
                for j in range(2):
                    kt = kvb * 2 + j
                    nc.tensor.matmul(
                        scp[:, j * 512:(j + 1) * 512],
                        kT_sb[kt // 4][lo:lo + 64, pt, (kt % 4) * 128:(kt % 4 + 1) * 128],
                        qT_sb[qc][lo:lo + 64, pt, :],
                        start=True,
                        stop=True,
                    )
                ex = ep.tile([128, 1024], dt.bfloat16, tag="ex",
                             name=f"ex_{qc}_{kvb}_{h}")
                nc.scalar.activation(ex[:], scp[:], AF.Exp, scale=0.125)
                return ex

            def attn(qc, kvb, h, ex):
                if qc not in psA:
                    psA[qc] = [psp.tile([128, 8, HD], dt.float32, tag="ps",
                                        name=f"att_{qc}_{b}") for b in range(2)]
                    psS[qc] = psp.tile([128, 16], dt.float32, tag="ps",
                                       name=f"asum_{qc}")
                # start=True zeroes the whole 2KB PSUM bank, so with several
                # accumulation groups per bank only the very first write into
                # each bank may carry start; everything else accumulates.
                for j in range(2):
                    kt = kvb * 2 + j
                    first = kvb == 0 and j == 0 and h == 0
                    last = kvb == 7 and j == 1 and h == HC - 1
                    for qt in range(4):
                        ex_sl = ex[:, j * 512 + qt * 128: j * 512 + (qt + 1) * 128]
                        nc.tensor.matmul(
                            psA[qc][qt // 2][:, (qt % 2) * 4 + h, :],
                            ex_sl,
                            v_sb[:, kt, h * 65:h * 65 + 64],
                            start=(first and qt % 2 == 0),
                            stop=(last and qt % 2 == 1),
                            skip_group_check=True,
                        )
                        c = qt * 4 + h
                        nc.tensor.matmul(
                            psS[qc][:, c:c + 1],
                            ex_sl,
                            v_sb[:, kt, h * 65 + 64:h * 65 + 65],
                            start=(first and qt == 0),
                            stop=(last and qt == 3),
                            skip_group_check=True,
                        )

            def tail_norm(qc):
                at = atp.tile([128, 2, 512], dt.bfloat16, tag="at", name=f"atT_{qc}")
                aqs = []
                for qt in range(4):
                    rc = rcp.tile([128, HC], dt.float32, tag="rc",
                                  name=f"rc_{qc}_{qt}")
                    nc.vector.reciprocal(rc[:], psS[qc][:, qt * 4:qt * 4 + 4])
                    aq_t = aqp.tile([128, DC], dt.bfloat16, tag="aq",
                                    name=f"aq_{qc}_{qt}")
                    for h in range(HC):
                        nc.vector.tensor_scalar_mul(
                            aq_t[:, h * 64:(h + 1) * 64],
                            psA[qc][qt // 2][:, (qt % 2) * 4 + h, :],
                            rc[:, h:h + 1],
                        )
                    if not drain_mode[0]:
                        for hp in range(2):
                            nc.sync.dma_start_transpose(
                                at[:, hp, qt * 128:(qt + 1) * 128],
                                aq_t[:, hp * 128:(hp + 1) * 128],
                            )
                    else:
                        aqs.append(aq_t)
                if drain_mode[0]:
                    # tail: PE is idle and HWDGE is busy with out-DMAs, so
                    # transpose via the PE (identity matmul) and copy the
                    # bf16 PSUM result back on the idle Pool engine
                    for qt in range(4):
                        for hp in range(2):
                            trp = pa.tile([128, 128], dt.bfloat16, tag="pa",
                                          name=f"trp_{qt}_{hp}")
                            nc.tensor.transpose(
                                trp[:], aqs[qt][:, hp * 128:(hp + 1) * 128],
                                ident_sb[:])
                            nc.vector.tensor_copy(
                                at[:, hp, qt * 128:(qt + 1) * 128], trp[:])
                del psA[qc]
                del psS[qc]
                atT[qc] = at

            def po_half(qc, qt, dc2, pool):
                poh = pool.tile([128, 512], dt.float32, tag=pool.name,
                                name=f"po_{qc}_{qt}_{dc2}")
                for hp in range(2):
                    nc.tensor.matmul(
                        poh[:],
                        atT[qc][:, hp, qt * 128:(qt + 1) * 128],
                        wo2_sb[:, hp, dc2 * 512:(dc2 + 1) * 512],
                        start=(hp == 0),
                        stop=(hp == 1),
                    )
                o_sb = op_.tile([128, 512], dt.float32, tag="o",
                                name=f"o_{qc}_{qt}_{dc2}")
                nc.vector.tensor_copy(o_sb[:], poh[:])
                nc.sync.dma_start(out_t[qc * 4 + qt][:, dc2 * 512:(dc2 + 1) * 512],
                                  o_sb[:])

            # ---- prologue PE work: pt0 pair only; the first two stream
            # tiles (h0/h1 read pt0) are emitted before the pt1 pair so their
            # exps do not absorb the pt1 projections into their waits
            qk_gran(k_rhs[0], wk_sb, kT_sb, bk_sb, 0, 0, pa)
            qk_gran(q_rhs[0], wq_sb, qT_sb, bq_sb, 0, 0, pa)

            # ---- filler queues for the budget scheduler ----
            # (nb = earliest stream-tile index; chosen from the DMA schedule)
            def kg(c, pt):
                return lambda: qk_gran(k_rhs[c], wk_sb, kT_sb, bk_sb, c, pt, ptp)

            def qg(c, pt):
                return lambda: qk_gran(q_rhs[c], wq_sb, qT_sb, bq_sb, c, pt, ptp)

            _LOG = []   # scheduler trace (debug)
            projq = [(2, 7, 1712.0, kg(1, 0)), (3, 9, 1712.0, kg(1, 1)),
                     (7, 15, 1712.0, kg(2, 0)), (8, 17, 1712.0, kg(2, 1)),
                     (11, 23, 1712.0, kg(3, 0)), (12, 25, 1712.0, kg(3, 1)),
                     (17, 31, 1712.0, qg(1, 0)), (18, 33, 1712.0, qg(1, 1)),
                     (40, 63, 1712.0, qg(2, 0)), (42, 65, 1712.0, qg(2, 1)),
                     (44, 95, 1712.0, qg(3, 0)), (46, 97, 1712.0, qg(3, 1))]
            vpq = [(12 if st2 < 4 else 21, 1712.0, st2) for st2 in range(ST // 2)]
            poq = []
            po_n = [0]
            attnq = []   # ("attn", nb, qc, kvb, h, ex) / ("tail", nb, qc) /
                         # ("po", nb, qc, qt, dc2)
            n_attn_emitted = [0]
            tr_free = [0]        # next stream idx the transient ring is free
            last_tail = [-10]    # stream idx of the last tail_norm pop
            drain_mode = [False]

            def attn_ready(item, idx):
                kind = item[0]
                if item[1] > idx:
                    return False
                if kind == "attn":
                    if item[2] not in psA and idx < last_tail[0] + 3:
                        return False
                    return vp_done[0] >= min(2 * item[3] + 2, ST)
                return True

            def pop_attn(idx):
                item = attnq.pop(0)
                _LOG.append((idx, item[0]))
                if item[0] == "attn":
                    _, _, qc, kvb, h, ex = item
                    attn(qc, kvb, h, ex)
                    n_attn_emitted[0] += 1
                    return 230.0
                qc = item[2]
                tail_norm(qc)
                last_tail[0] = idx
                for k, (qt, dc2) in enumerate(
                        (q, d) for q in range(4) for d in range(2)):
                    poq.append((idx + 4 + 2 * k, qc, qt, dc2))
                return 0.0

            def pop_po(idx):
                nb, qc, qt, dc2 = poq.pop(0)
                _LOG.append((idx, "po"))
                if drain_mode[0]:
                    pool = pa if po_n[0] % 2 == 0 else ptp
                else:
                    pool = ptp
                po_n[0] += 1
                po_half(qc, qt, dc2, pool)
                tr_free[0] = idx + 2
                return 426.0

            carry = [0.0]
            BUDGET, CAP, FORCE_AT = 550.0, 2600.0, 19

            def fillers(idx, force_at=FORCE_AT):
                carry[0] = min(carry[0] + BUDGET, CAP)
                while True:
                    pending = idx + 1 - n_attn_emitted[0]
                    tr_ok = idx >= tr_free[0]
                    if poq and tr_ok and idx >= poq[0][0] + 6:
                        carry[0] -= pop_po(idx)
                        continue
                    if pending >= force_at and attnq:
                        it = attnq[0]
                        if (it[0] == "attn"
                                and vp_done[0] < min(2 * it[3] + 2, ST) and vpq):
                            nb, cost, st2 = vpq.pop(0)
                            v_proj2(st2)
                            carry[0] -= cost
                            tr_free[0] = idx + 3
                            continue
                        if attn_ready(it, idx) or pending >= force_at + 6:
                            carry[0] -= pop_attn(idx)
                            continue
                    # cheap attn/tail pops first: their tiny matmuls shield
                    # the preceding scores tile's exp from absorbing a
                    # transient's matmuls into its coalesced semaphore wait
                    if attnq and attn_ready(attnq[0], idx):
                        kind = attnq[0][0]
                        if kind == "attn" and 230.0 <= carry[0]:
                            carry[0] -= pop_attn(idx)
                            continue
                        if kind == "tail":
                            carry[0] -= pop_attn(idx)
                            continue
                    if (projq and projq[0][0] <= idx
                            and projq[0][2] <= carry[0] and tr_ok):
                        nb, dl, cost, fn = projq.pop(0)
                        fn()
                        _LOG.append((idx, "proj"))
                        carry[0] -= cost
                        tr_free[0] = idx + 3
                        continue
                    if (vpq and vpq[0][0] <= idx
                            and vpq[0][1] <= carry[0] and tr_ok):
                        nb, cost, st2 = vpq.pop(0)
                        v_proj2(st2)
                        carry[0] -= cost
                        tr_free[0] = idx + 3
                        continue
                    if poq and poq[0][0] <= idx and 426.0 <= carry[0] and tr_ok:
                        carry[0] -= pop_po(idx)
                        continue
                    break

            # ---- the exp stream with interleaved fillers ----
            def prologue_pt1(idx):
                qk_gran(k_rhs[0], wk_sb, kT_sb, bk_sb, 0, 1, ptp)
                qk_gran(q_rhs[0], wq_sb, qT_sb, bq_sb, 0, 1, ptp)
                tr_free[0] = idx + 4

            idx = 0
            for qc in range(QC):
                for kvb in range(8):
                    for h in range(HC):
                        if idx == 2:
                            prologue_pt1(idx)
                        # hard deadline: a projection chunk must be emitted
                        # before the first scores tile that reads it
                        while projq and projq[0][1] <= idx:
                            nb, dl, cost, fn = projq.pop(0)
                            fn()
                            carry[0] -= cost
                            tr_free[0] = idx + 3
                        ex = scores_exp(qc, kvb, h)
                        attnq.append(("attn", idx + 1, qc, kvb, h, ex))
                        if kvb == 7 and h == HC - 1:
                            attnq.append(("tail", idx + 2, qc))
                        fa = FORCE_AT
                        if qc == QC - 1:
                            fa = max(2, FORCE_AT - max(0, idx - 96))
                        fillers(idx, fa)
                        idx += 1

            # ---- drain: remaining attn, last tail, last po (pa ring is free
            # now, so po ping-pongs through it instead of the 1-bank ring)
            drain_mode[0] = True
            _LOG.append(("DRAIN", [it[0] for it in attnq], len(poq),
                         len(projq), len(vpq)))
            while projq or vpq or attnq or poq:
                if projq:
                    projq.pop(0)[3]()
                    continue
                if vpq:
                    v_proj2(vpq.pop(0)[2])
                    continue
                if attnq and attn_ready(attnq[0], 10 ** 9):
                    pop_attn(idx)
                    idx += 1
                    continue
                if poq:
                    pop_po(idx)
                    idx += 1
                    continue
                raise RuntimeError("scheduler deadlock")

    nc.finalize()
    return nc


def _get_program():
    global _PROGRAM
    if _PROGRAM is None:
        _PROGRAM = _build_program()
    return _PROGRAM


def _prep_core_inputs(x_q, x_k, x_v, wq, bq, wk, bk, wv, bv, wo):
    """Build the 8 per-core input dicts (host-side shard + cast)."""
    xT = {}
    for b in range(2):
        xT[b] = (
            np.ascontiguousarray(x_q[b].T).astype(BF16),
            np.ascontiguousarray(x_k[b].T).astype(BF16),
            np.ascontiguousarray(x_v[b].T).astype(BF16),
        )
    in_maps = []
    for c in range(NCORES):
        b, g = c // 4, c % 4
        sl = slice(g * DC, (g + 1) * DC)
        # wo2[p, hp, :] = wo[g*DC + hp*128 + p, :] - head pairs stacked to 128
        # partitions, matching the transposed at_q column order.
        wo2_c = np.ascontiguousarray(
            wo[sl, :].reshape(2, 128, D).transpose(1, 0, 2)
        ).astype(BF16)
        in_maps.append({
            "xqT": xT[b][0],
            "xkT": xT[b][1],
            "xvT": xT[b][2],
            "wq": wq[:, sl].astype(BF16),
            "wk": wk[:, sl].astype(BF16),
            "wv": wv[:, sl].astype(BF16),
            "wo2": wo2_c,
            "bq": np.ascontiguousarray(bq[sl].reshape(2, 128).T).astype(np.float32),
            "bk": np.ascontiguousarray(bk[sl].reshape(2, 128).T).astype(np.float32),
            "bv": np.broadcast_to(bv[sl], (128, DC)).astype(np.float32).copy(),
            "ident": np.eye(128, dtype=BF16),
        })
    return in_maps


def kernel(x_q, x_k, x_v, wq, bq, wk, bk, wv, bv, wo, bo):
    from concourse.bass_utils import run_bass_kernel_spmd

    x_q = np.asarray(x_q, np.float32)
    x_k = np.asarray(x_k, np.float32)
    x_v = np.asarray(x_v, np.float32)
    wq = np.asarray(wq, np.float32)
    wk = np.asarray(wk, np.float32)
    wv = np.asarray(wv, np.float32)
    wo = np.asarray(wo, np.float32)
    bq = np.asarray(bq, np.float32)
    bk = np.asarray(bk, np.float32)
    bv = np.asarray(bv, np.float32)
    bo = np.asarray(bo, np.float32)

    nc = _get_program()
    in_maps = _prep_core_inputs(x_q, x_k, x_v, wq, bq, wk, bk, wv, bv, wo)
    res = run_bass_kernel_spmd(nc, in_maps, list(range(NCORES)))

    out = np.zeros((2, S, D), np.float32)
    for c in range(NCORES):
        out[c // 4] += res.results[c]["out"]
    out += bo
    return out
